# revision 14
# baseline (speedup 1.0000x reference)
"""Sobel filter Trainium2 Bass kernel.

Problem: img [32, 3, 512, 512] f32, kx/ky [1, 3, 3, 3] f32 (same 3x3 kernel
broadcast over the 3 input channels in the reference, but we honor arbitrary
values). Output [32, 1, 512, 512] f32:
    Gx = valid_conv3x3(img, kx), Gy = valid_conv3x3(img, ky)  -> [N,1,510,510]
    out = sqrt(Gx^2 + Gy^2) edge-padded by 1 back to [N,1,512,512]

Strategy (pure data parallel over 8 NeuronCores, 4 images per core):
  The 2D conv runs on the TensorEngine as sums of banded-Toeplitz matmuls.
  Partition dim = image rows (y). For each (channel c, x-shift dx) the 3-tap
  y-convolution is a banded [K=128, M=126] stationary matrix
  A[k, m] = w[c, k-m, dx]; the moving operand is the x-shifted image rows
  img[c, y0:y0+128, dx:dx+510]. Summing over (c, dx) for each of Gx/Gy is
  PSUM accumulation over 9 matmuls -> [126, 510] valid conv rows per PSUM
  tile. 4 row-tiles of 126 cover rows 0..503; the remaining 6 valid rows of
  ALL 4 images are computed by one extra "mini" tile with a block-diagonal
  [32, 24] stationary (4 blocks of [8 in-rows, 6 out-rows]).

  Matmul operands are float32r (full-rate fp32 matmul mode; plain float32
  streams at 1/4 rate). Loads use 128-partition DMAs (104-partition DMAs
  measured at 159 GB/s vs 286 GB/s for 128). Magnitude epilogue: squares on
  ScalarE (PSUM->SBUF), add on VectorE, sqrt on ScalarE; column edge padding
  in-SBUF, row edge padding via small extra stores.

The banded stationary matrices (built from kx/ky on host) are passed as
replicated input tensors.
"""

import os

import numpy as np

N_CORES = 8
N_FULL = 32          # full batch
N_PER_CORE = N_FULL // N_CORES
H = W = 512
TILE_K = 128         # input rows per full row-tile
TILE_M = 126         # valid output rows per full row-tile
N_TILES = 4          # 4 * 126 = 504 valid rows; remaining 6 via mini tile
NW = 510             # valid output columns
MINI_K = 8 * N_PER_CORE   # 4 images x 8 input rows
MINI_M = 6 * N_PER_CORE   # 4 images x 6 output rows

_CACHE: dict = {}
LAST_RESULTS = None  # BassKernelResults of the most recent run (for test.py)


def _build_stationaries(kx: np.ndarray, ky: np.ndarray):
    """Returns (stat [TILE_K, 18, TILE_M], stat_mini [MINI_K, 18, MINI_M]).
    Slice i=(g,c,dx) of stat is the banded matrix A[k, m] = kG[c, k-m, dx]
    for k-m in {0,1,2}; stat_mini is block-diagonal per image."""
    ks = (np.asarray(kx, np.float32), np.asarray(ky, np.float32))
    stat = np.zeros((18, TILE_K, TILE_M), np.float32)
    mini = np.zeros((18, MINI_K, MINI_M), np.float32)
    m = np.arange(TILE_M)
    mm = np.arange(6)
    i = 0
    for g in range(2):
        for c in range(3):
            for dx in range(3):
                for dy in range(3):
                    stat[i, m + dy, m] = ks[g][0, c, dy, dx]
                    for j in range(N_PER_CORE):
                        mini[i, j * 8 + mm + dy, j * 6 + mm] = ks[g][0, c, dy, dx]
                i += 1
    return (
        np.ascontiguousarray(stat.transpose(1, 0, 2)),
        np.ascontiguousarray(mini.transpose(1, 0, 2)),
    )


def _epilogue(nc, work_pool, psx, psy, rows, f32):
    """sqrt(psx^2 + psy^2) -> [rows, 512] SBUF tile with edge cols."""
    s = work_pool.tile([rows, W], f32, tag="s", name="s")
    s2 = work_pool.tile([rows, NW], f32, tag="s2", name="s2")
    nc.scalar.square(s[:, 1 : 1 + NW], psx)
    nc.scalar.square(s2, psy)
    nc.vector.tensor_add(s[:, 1 : 1 + NW], s[:, 1 : 1 + NW], s2)
    nc.vector.tensor_copy(s[:, 0:1], s[:, 1:2])
    nc.vector.tensor_copy(s[:, W - 1 : W], s[:, W - 2 : W - 1])
    mag = work_pool.tile([rows, W], f32, tag="mag", name="mag")
    nc.scalar.sqrt(mag, s)
    return mag


def _sobel_body(tc, out, img, stat_dram, stat_mini_dram):
    import concourse.mybir as mybir

    nc = tc.nc
    f32 = mybir.dt.float32
    mm_dt = mybir.dt.float32r

    img_yx = img.rearrange("n c y x -> n y c x")

    with (
        tc.tile_pool(name="const", bufs=1) as const_pool,
        tc.tile_pool(name="imgs", bufs=3) as img_pool,
        tc.tile_pool(name="work", bufs=3) as work_pool,
        tc.tile_pool(name="psum", bufs=2, space="PSUM") as psum_pool,
    ):
        # stat loads split into piece-pairs in MM order: the first big matmul
        # only waits for pieces (0, 1), which go on the sync ring ahead of the
        # image loads; the rest go on the scalar ring (idle early; stores
        # appear there only later).
        stat_sb = const_pool.tile([TILE_K, 18, TILE_M], mm_dt)
        nc.sync.dma_start(out=stat_sb[:, 0:2], in_=stat_dram[:, 0:2])
        for j in range(1, 9):
            nc.scalar.dma_start(
                out=stat_sb[:, 2 * j : 2 * j + 2], in_=stat_dram[:, 2 * j : 2 * j + 2]
            )
        stat_mini_sb = const_pool.tile([MINI_K, 18, MINI_M], mm_dt)
        nc.scalar.dma_start(out=stat_mini_sb, in_=stat_mini_dram)
        # mini-tile inputs on the scalar ring: these 8-partition DMAs are
        # slow (~2-4us each) and would stall the load ring mid-stream
        mit = img_pool.tile([MINI_K, 3, W], mm_dt, tag="mit", bufs=1)
        for j in range(N_PER_CORE):
            nc.scalar.dma_start(
                out=mit[j * 8 : (j + 1) * 8], in_=img_yx[j, H - 8 : H]
            )

        def big_tile(n, t):
            y0 = t * TILE_M
            # per-channel loads -> finer-grained MM/DMA pipelining. All loads
            # on the sync HWDGE ring, all stores on the scalar ring: measured
            # 287 GB/s vs 215 GB/s with loads+stores sharing a ring.
            its = []
            for c in range(3):
                itc = img_pool.tile(
                    [TILE_K, W], mm_dt, tag=f"it{c}", name=f"it{c}", bufs=4
                )
                nc.sync.dma_start(out=itc, in_=img_yx[n, y0 : y0 + TILE_K, c])
                its.append(itc)

            psx = psum_pool.tile([TILE_M, NW], f32, tag="psx", name="psx")
            psy = psum_pool.tile([TILE_M, NW], f32, tag="psy", name="psy")
            for g, ps in ((0, psx), (1, psy)):
                mmi = 0
                for c in range(3):
                    for dx in range(3):
                        i = (g * 3 + c) * 3 + dx
                        nc.tensor.matmul(
                            ps,
                            stat_sb[:, i, :],
                            its[c][:, dx : dx + NW],
                            start=(mmi == 0),
                            stop=(mmi == 8),
                        )
                        mmi += 1

            mag = _epilogue(nc, work_pool, psx, psy, TILE_M, f32)
            nc.scalar.dma_start(out=out[n, 1 + y0 : 1 + y0 + TILE_M, :], in_=mag)
            if t == 0:
                nc.scalar.dma_start(out=out[n, 0:1, :], in_=mag[0:1, :])

        def mini_tile():
            # last 6 valid rows (y' = 504..509) of all 4 images at once,
            # via a block-diagonal stationary
            mpsx = psum_pool.tile([MINI_M, NW], f32, tag="mpsx", bufs=1, name="mpsx")
            mpsy = psum_pool.tile([MINI_M, NW], f32, tag="mpsy", bufs=1, name="mpsy")
            for g, ps in ((0, mpsx), (1, mpsy)):
                mmi = 0
                for c in range(3):
                    for dx in range(3):
                        i = (g * 3 + c) * 3 + dx
                        nc.tensor.matmul(
                            ps,
                            stat_mini_sb[:, i, :],
                            mit[:, c, dx : dx + NW],
                            start=(mmi == 0),
                            stop=(mmi == 8),
                        )
                        mmi += 1
            mmag = _epilogue(nc, work_pool, mpsx, mpsy, MINI_M, f32)
            for n in range(N_PER_CORE):
                nc.scalar.dma_start(
                    out=out[n, H - 7 : H - 1, :], in_=mmag[n * 6 : n * 6 + 6]
                )
                nc.scalar.dma_start(
                    out=out[n, H - 1 : H, :], in_=mmag[n * 6 + 5 : n * 6 + 6]
                )

        done_mini = False
        for n in range(N_PER_CORE):
            for t in range(N_TILES):
                big_tile(n, t)
                if n == 1 and t == 0 and not done_mini:
                    mini_tile()
                    done_mini = True


def _build_program():
    import concourse.bacc as bacc
    import concourse.mybir as mybir
    import concourse.tile as tile

    nc = bacc.Bacc(
        "TRN2",
        target_bir_lowering=False,
        debug=False,
        num_devices=N_CORES,
    )
    img = nc.dram_tensor(
        "img", [N_PER_CORE, 3, H, W], mybir.dt.float32r, kind="ExternalInput"
    ).ap()
    stat = nc.dram_tensor(
        "stat", [TILE_K, 18, TILE_M], mybir.dt.float32r, kind="ExternalInput"
    ).ap()
    stat_mini = nc.dram_tensor(
        "stat_mini", [MINI_K, 18, MINI_M], mybir.dt.float32r, kind="ExternalInput"
    ).ap()
    out = nc.dram_tensor(
        "out", [N_PER_CORE, H, W], mybir.dt.float32, kind="ExternalOutput"
    ).ap()

    with tile.TileContext(nc) as tc:
        _sobel_body(tc, out, img, stat, stat_mini)
    nc.compile()
    return nc


def kernel(img: np.ndarray, kx: np.ndarray, ky: np.ndarray) -> np.ndarray:
    global LAST_RESULTS
    from concourse.bass_utils import run_bass_kernel_spmd

    img = np.ascontiguousarray(np.asarray(img, dtype=np.float32))
    assert img.shape == (N_FULL, 3, H, W), img.shape
    stat, stat_mini = _build_stationaries(kx, ky)

    if "nc" not in _CACHE:
        _CACHE["nc"] = _build_program()
    nc = _CACHE["nc"]

    in_maps = [
        {
            "img": img[c * N_PER_CORE : (c + 1) * N_PER_CORE],
            "stat": stat,
            "stat_mini": stat_mini,
        }
        for c in range(N_CORES)
    ]
    trace = os.environ.get("SOBEL_TRACE", "0") == "1"
    res = run_bass_kernel_spmd(
        nc, in_maps, core_ids=list(range(N_CORES)), trace=trace
    )
    LAST_RESULTS = res
    out = np.concatenate([res.results[c]["out"] for c in range(N_CORES)], axis=0)
    return out.reshape(N_FULL, 1, H, W)
